# revision 6
# baseline (speedup 1.0000x reference)
"""Trainium2 Bass kernel for nn_CNN (char-CNN embedding + 5-tap conv + maxpool).

Computation (per reference):
  emb[b,s,w,:]  = char_table[char_ids[b,s,w], :]        (gather, 512x64 table)
  emb          *= (s < length[b])                        (word mask)
  conv[b,s,m,:] = conv_b + sum_k conv_w[k]*emb[b,s,m+k-2,:]  ('same' pad)
  out[b,s,:]    = max_m conv[b,s,m,:]
Returns (out, length).

Device strategy (pure data parallel over batch, 8 cores x 32 batches):
  - Indices of invalid words are bumped by +512 into an all-zero region of a
    [1024, 64] padded table, so masked words gather zeros -> conv = 0 ->
    max = 0 -> out = bias.  (Mask compare runs on DVE from device-side
    length/s-index tiles.)
  - Gather: per-slot indirect DMA.  Slot s_ holds 8 words x 16 chars on the
    128 partitions; one indirect_dma_start per slot pulls row idx[p, s_]
    into gbuf[p, s_, :].
  - PE computes the conv as out[d, word*16+m] = emb_slot[wc, d].T @ C[wc, .]
    with C the block-diagonal banded conv matrix (gathered tile is the
    stationary operand so output lands d-on-partitions, positions-on-free).
  - DVE reduce_max over the 16 positions (innermost free axis), then adds
    bias.  Host inverts the layout and concatenates cores.
"""

import os
import sys

os.environ.setdefault("MYCRO_LOCAL_CACHE", "1")
for _p in ("/opt/trn_rl_repo", "/root/.axon_site/_ro/trn_rl_repo"):
    if os.path.isdir(_p) and _p not in sys.path:
        sys.path.insert(0, _p)
        break

import numpy as np

B, S, W, D, V = 256, 64, 16, 64, 512
KTAP, PADW = 5, 2
NCORES = 8
BPC = B // NCORES            # 32 batches per core
WORDS = BPC * S              # 2048 words per core
TOK = WORDS * W              # 32768 gather tokens per core
SLOTS = TOK // 128           # 256 slots (8 words each)
NBANK = 8                    # psum banks in flight
ROUNDS = SLOTS // 8          # 32 psum rounds (8 slots / 64 words each)

_CACHE = {}


def _build_nc():
    import concourse.bass as bass
    import concourse.mybir as mybir

    f32, i32 = mybir.dt.float32, mybir.dt.int32
    X = mybir.AxisListType.X
    Alu = mybir.AluOpType

    nc = bass.Bass()

    table = nc.declare_dram_parameter("table", [2 * V, D], f32, isOutput=False)
    idx_in = nc.declare_dram_parameter("idx", [128, SLOTS], i32, isOutput=False)
    sidx_in = nc.declare_dram_parameter("sidx", [128, SLOTS], i32, isOutput=False)
    len_in = nc.declare_dram_parameter("lens", [128, SLOTS], i32, isOutput=False)
    cmat_in = nc.declare_dram_parameter("cmat", [128, 128], f32, isOutput=False)
    bias_in = nc.declare_dram_parameter("bias", [128, 1], f32, isOutput=False)
    out_t = nc.declare_dram_parameter("out", [128, WORDS // 2], f32, isOutput=True)

    with (
        nc.sbuf_tensor("idx_sb", [128, SLOTS], i32) as idx_sb,
        nc.sbuf_tensor("sidx_sb", [128, SLOTS], i32) as sidx_sb,
        nc.sbuf_tensor("len_sb", [128, SLOTS], i32) as len_sb,
        nc.sbuf_tensor("cmat_sb", [128, 128], f32) as cmat_sb,
        nc.sbuf_tensor("bias_sb", [128, 1], f32) as bias_sb,
        nc.sbuf_tensor("gbuf", [128, SLOTS, D], f32) as gbuf,
        nc.sbuf_tensor("res", [128, WORDS // 2], f32) as res,
        nc.psum_tensor("ps", [128, NBANK, 512], f32) as ps,
        nc.semaphore("in_sem") as in_sem,
        nc.semaphore("idx_sem") as idx_sem,
        nc.semaphore("gat_sem") as gat_sem,
        nc.semaphore("mm_sem") as mm_sem,
        nc.semaphore("red_sem") as red_sem,
        nc.semaphore("fin_sem") as fin_sem,
        nc.Block() as block,
    ):

        @block.sync
        def _(sp):
            sp.dma_start(out=idx_sb[:], in_=idx_in[:]).then_inc(in_sem, 16)
            sp.dma_start(out=sidx_sb[:], in_=sidx_in[:]).then_inc(in_sem, 16)
            sp.dma_start(out=len_sb[:], in_=len_in[:]).then_inc(in_sem, 16)
            sp.dma_start(out=cmat_sb[:], in_=cmat_in[:]).then_inc(in_sem, 16)
            sp.dma_start(out=bias_sb[:], in_=bias_in[:]).then_inc(in_sem, 16)
            sp.wait_ge(fin_sem, 1)
            sp.dma_start(out=out_t[:], in_=res[:]).then_inc(in_sem, 16)
            sp.wait_ge(in_sem, 96)

        @block.gpsimd
        def _(g):
            g.wait_ge(idx_sem, 1)
            for s_ in range(SLOTS):
                g.indirect_dma_start(
                    out=gbuf[:, s_, :],
                    out_offset=None,
                    in_=table[:, :],
                    in_offset=bass.IndirectOffsetOnAxis(
                        ap=idx_sb[:, s_ : s_ + 1], axis=0
                    ),
                ).then_inc(gat_sem, 16)

        @block.tensor
        def _(t):
            t.wait_ge(in_sem, 80)
            for r in range(ROUNDS):
                t.wait_ge(gat_sem, 16 * 8 * (r + 1))
                if r >= NBANK:
                    t.wait_ge(red_sem, r - NBANK + 1)
                bank = ps[:, r % NBANK, :]
                mm = None
                for q in range(4):
                    for par in range(2):
                        s_ = r * 8 + q * 2 + par
                        mm = nc.tensor.matmul(
                            bank[par * 64 : (par + 1) * 64, q * 128 : (q + 1) * 128],
                            gbuf[:, s_, :],
                            cmat_sb[:, :],
                            start=True,
                            stop=True,
                        )
                mm.then_inc(mm_sem, 1)

        @block.vector
        def _(v):
            v.wait_ge(in_sem, 80)
            # ge = (s_of_word >= length_of_word)  (1/0), then idx += 512*ge
            # -> masked words gather the all-zero rows [512, 1024).
            v.tensor_tensor(out=len_sb[:], in0=sidx_sb[:], in1=len_sb[:], op=Alu.is_ge)
            v.scalar_tensor_tensor(
                out=idx_sb[:],
                in0=len_sb[:],
                scalar=512,
                in1=idx_sb[:],
                op0=Alu.mult,
                op1=Alu.add,
            ).then_inc(idx_sem, 1)
            for r in range(ROUNDS):
                v.wait_ge(mm_sem, r + 1)
                rin = ps[:, r % NBANK, :].rearrange("p (q w m) -> p q w m", q=4, w=8, m=16)
                v.reduce_max(res[:, r * 32 : (r + 1) * 32], rin, axis=X).then_inc(
                    red_sem, 1
                )
            v.tensor_scalar_add(res[:], res[:], bias_sb[:, :]).then_inc(fin_sem, 1)

    return nc


def _host_inputs(char_ids, length, char_table, conv_w, conv_b):
    """Build the 8 per-core input maps (pure layout/replication on host)."""
    table_pad = np.concatenate(
        [np.asarray(char_table, np.float32), np.zeros((V, D), np.float32)], axis=0
    )
    cmat = np.zeros((128, 128), np.float32)
    cw = np.asarray(conv_w, np.float32)
    for w in range(8):
        for cc in range(16):
            for m in range(16):
                k = cc - m + 2
                if 0 <= k < KTAP:
                    cmat[w * 16 + cc, w * 16 + m] = cw[k]
    bias128 = np.full((128, 1), np.asarray(conv_b, np.float32)[0], np.float32)

    # g_mat[p, s_] = global word index of (partition p, slot s_) within a core
    p_arr = np.arange(128)
    s_arr = np.arange(SLOTS)
    g_mat = s_arr[None, :] * 8 + (p_arr // 16)[:, None]          # [128, SLOTS]
    sidx = (g_mat % S).astype(np.int32)

    ids = np.asarray(char_ids, np.int32)
    lens_all = np.asarray(length, np.int32)
    in_maps = []
    for c in range(NCORES):
        idsc = ids[c * BPC : (c + 1) * BPC].reshape(WORDS, W)     # [g, c]
        # idx[p, s_] = char (p%16) of word g(p, s_)
        idx = np.ascontiguousarray(
            idsc.reshape(SLOTS, 8, W).transpose(1, 2, 0).reshape(128, SLOTS)
        ).astype(np.int32)
        lensc = lens_all[c * BPC : (c + 1) * BPC][g_mat // S].astype(np.int32)
        in_maps.append(
            {
                "table": table_pad,
                "idx": idx,
                "sidx": sidx,
                "lens": lensc,
                "cmat": cmat,
                "bias": bias128,
            }
        )
    return in_maps


def _unshard(results, length):
    outs = []
    for c in range(NCORES):
        res = np.asarray(results[c]["out"])  # [128, 1024]
        data = (
            res.reshape(2, 64, ROUNDS, 4, 8)
            .transpose(2, 3, 0, 4, 1)
            .reshape(BPC, S, D)
        )
        outs.append(data)
    return np.concatenate(outs, axis=0), np.asarray(length)


def _install_ntff_hook():
    """The agent image's antenv lacks axon_hooks; recreate it so
    run_bass_kernel_spmd(trace=True) can capture NTFF profiles."""
    import types

    if "antenv.axon_hooks" in sys.modules:
        return
    import antenv

    mod = types.ModuleType("antenv.axon_hooks")
    mod._hook = None
    mod.set_axon_ntff_profile_hook = lambda h: setattr(mod, "_hook", h)
    mod.get_axon_ntff_profile_hook = lambda: mod._hook
    sys.modules["antenv.axon_hooks"] = mod
    antenv.axon_hooks = mod
    try:
        if "/root/.axon_site" not in sys.path:
            sys.path.append("/root/.axon_site")
        from trn_agent_boot.trn_boot import _ntff_profile_via_ctypes

        hook = _ntff_profile_via_ctypes("/opt/axon/libaxon_pjrt.so")
        if hook is not None:
            mod.set_axon_ntff_profile_hook(hook)
    except Exception as e:  # degrade to no tracing
        print(f"ntff hook install failed: {e}")
    # upload_artifacts needs bucket creds we don't have; skip it.
    import concourse.bass_utils as bu

    bu.upload_artifacts = lambda tmpdir: f"local://{tmpdir}"


def _run(in_maps, trace=False, tmpdir=None):
    from concourse.bass_utils import run_bass_kernel_spmd

    if "nc" not in _CACHE:
        _CACHE["nc"] = _build_nc()
    nc = _CACHE["nc"]
    if trace:
        _install_ntff_hook()
    return run_bass_kernel_spmd(
        nc, in_maps, list(range(NCORES)), trace=trace, tmpdir=tmpdir
    )


def kernel(char_ids, length, char_table, conv_w, conv_b):
    in_maps = _host_inputs(char_ids, length, char_table, conv_w, conv_b)
    br = _run(in_maps, trace=False)
    data, length_out = _unshard(br.results, length)
    return data, length_out


def kernel_profiled(char_ids, length, char_table, conv_w, conv_b, tmpdir=None):
    """Like kernel() but with NTFF tracing; returns (output, exec_time_ns)."""
    in_maps = _host_inputs(char_ids, length, char_table, conv_w, conv_b)
    br = _run(in_maps, trace=True, tmpdir=tmpdir)
    data, length_out = _unshard(br.results, length)
    return (data, length_out), br.exec_time_ns


# revision 29
# speedup vs baseline: 1.3179x; 1.3179x over previous
"""Trainium2 Bass kernel for nn_CNN (char-CNN embedding + 5-tap conv + maxpool).

Computation (per reference):
  emb[b,s,w,:]  = char_table[char_ids[b,s,w], :]        (gather, 512x64 table)
  emb          *= (s < length[b])                        (word mask)
  conv[b,s,m,:] = conv_b + sum_k conv_w[k]*emb[b,s,m+k-2,:]  ('same' pad)
  out[b,s,:]    = max_m conv[b,s,m,:]
Returns (out, length).

Device strategy (pure data parallel over batch, 8 cores x 32 batches):
  - Indices of invalid words are bumped by +512 into an all-zero region of a
    [1024, 64] padded table, so masked words gather zeros -> conv = 0 ->
    max = 0 -> out = bias.  (Mask compare runs on DVE from device-side
    length/s-index tiles.)
  - Gather: per-slot indirect DMA.  Slot s_ holds 8 words x 16 chars on the
    128 partitions; one indirect_dma_start per slot pulls row idx[p, s_]
    into gbuf[p, s_, :].
  - PE computes the conv as out[d, word*16+m] = emb_slot[wc, d].T @ C[wc, .]
    with C the block-diagonal banded conv matrix (gathered tile is the
    stationary operand so output lands d-on-partitions, positions-on-free).
  - DVE reduce_max over the 16 positions (innermost free axis), then adds
    bias.  Host inverts the layout and concatenates cores.
"""

import os
import sys

os.environ.setdefault("MYCRO_LOCAL_CACHE", "1")
for _p in ("/opt/trn_rl_repo", "/root/.axon_site/_ro/trn_rl_repo"):
    if os.path.isdir(_p) and _p not in sys.path:
        sys.path.insert(0, _p)
        break

import numpy as np

B, S, W, D, V = 256, 64, 16, 64, 512
KTAP, PADW = 5, 2
NCORES = 8
BPC = B // NCORES            # 32 batches per core
WORDS = BPC * S              # 2048 words per core
TOK = WORDS * W              # 32768 gather tokens per core
SLOTS = TOK // 128           # 256 slots (8 words each)
NBANK = 8                    # psum banks in flight
ROUNDS = SLOTS // 8          # 32 psum rounds (8 slots / 64 words each)
NCHUNK = 8                   # gather chunks
SPC = SLOTS // NCHUNK        # 32 slots per gather chunk

_CACHE = {}


def _build_nc():
    from contextlib import ExitStack

    import concourse.bacc as bacc
    import concourse.bass as bass
    import concourse.mybir as mybir
    from concourse.library_config import mlp

    f32, i16 = mybir.dt.float32, mybir.dt.int16
    X = mybir.AxisListType.X
    Alu = mybir.AluOpType

    nc = bacc.Bacc("TRN2")

    NW = TOK // 16  # wrapped idx columns (= words per core)
    table = nc.declare_dram_parameter("table", [2 * V, D], f32, isOutput=False)
    idx_in = nc.declare_dram_parameter("idx", [128, NW], i16, isOutput=False)
    sidx_in = nc.declare_dram_parameter("sidx", [128, NW], i16, isOutput=False)
    len_in = nc.declare_dram_parameter("lens", [128, NW], i16, isOutput=False)
    cmat_in = nc.declare_dram_parameter("cmat", [128, 128], f32, isOutput=False)
    bias_in = nc.declare_dram_parameter("bias", [128, 1], f32, isOutput=False)
    out_t = nc.declare_dram_parameter("out", [128, WORDS // 2], f32, isOutput=True)

    with (
        nc.sbuf_tensor("idx_sb", [128, NW], i16) as idx_sb,
        nc.sbuf_tensor("sidx_sb", [128, NW], i16) as sidx_sb,
        nc.sbuf_tensor("len_sb", [128, NW], i16) as len_sb,
        nc.sbuf_tensor("cmat_sb", [128, 128], f32) as cmat_sb,
        nc.sbuf_tensor("bias_sb", [128, 1], f32) as bias_sb,
        nc.sbuf_tensor("gbuf", [128, SLOTS, D], f32) as gbuf,
        nc.sbuf_tensor("res", [128, WORDS // 2], f32) as res,
        nc.psum_tensor("ps", [128, NBANK, 512], f32) as ps,
        nc.semaphore("in_sem") as in_sem,
        nc.semaphore("msk_sem") as msk_sem,
        nc.semaphore("idx_sem") as idx_sem,
        nc.semaphore("mm_sem") as mm_sem,
        nc.semaphore("red_sem") as red_sem,
        nc.semaphore("fin_sem") as fin_sem,
        ExitStack() as stack,
        nc.Block() as block,
    ):
        gat_sems = [
            stack.enter_context(nc.semaphore(f"gat{c}")) for c in range(NCHUNK + 1)
        ]

        @block.sync
        def _(sp):
            sp.dma_start(out=idx_sb[:], in_=idx_in[:]).then_inc(in_sem, 16)
            sp.dma_start(out=sidx_sb[:], in_=sidx_in[:]).then_inc(in_sem, 16)
            sp.dma_start(out=len_sb[:], in_=len_in[:]).then_inc(in_sem, 16)
            sp.dma_start(out=cmat_sb[:], in_=cmat_in[:]).then_inc(in_sem, 16)
            sp.dma_start(out=bias_sb[:], in_=bias_in[:]).then_inc(in_sem, 16)
            sp.wait_ge(fin_sem, 1)
            sp.dma_start(out=out_t[:], in_=res[:]).then_inc(in_sem, 16)
            sp.wait_ge(in_sem, 96)

        IDX_PC = SPC * 128  # indices per gather chunk

        @block.gpsimd
        def _(g):
            g.load_library(mlp)
            g.wait_ge(idx_sem, 1)
            for c in range(NCHUNK):
                g.dma_gather(
                    gbuf[:, c * SPC : (c + 1) * SPC, :],
                    table[:, :],
                    idx_sb[:, c * (IDX_PC // 16) : (c + 1) * (IDX_PC // 16)],
                    IDX_PC,
                    IDX_PC,
                    D,
                    single_packet=False,
                ).then_inc(gat_sems[c], 16)

        @block.tensor
        def _(t):
            t.wait_ge(in_sem, 80)
            for r in range(ROUNDS):
                c = r // (ROUNDS // NCHUNK)
                t.wait_ge(gat_sems[c], 16)
                if r >= NBANK:
                    t.wait_ge(red_sem, r - NBANK + 1)
                bank = ps[:, r % NBANK, :]
                mm = None
                for q in range(4):
                    for par in range(2):
                        s_ = r * 8 + q * 2 + par
                        mm = nc.tensor.matmul(
                            bank[par * 64 : (par + 1) * 64, q * 128 : (q + 1) * 128],
                            gbuf[:, s_, :],
                            cmat_sb[:, :],
                            start=True,
                            stop=True,
                        )
                mm.then_inc(mm_sem, 1)

        @block.vector
        def _(v):
            v.wait_ge(in_sem, 80)
            # ge = (s_of_word >= length_of_word)  (1/0), then idx += 512*ge
            # -> masked words gather the all-zero rows [512, 1024).
            v.tensor_tensor(
                out=len_sb[:], in0=sidx_sb[:], in1=len_sb[:], op=Alu.is_ge
            ).then_inc(msk_sem, 1)
            v.wait_ge(msk_sem, 1)
            v.scalar_tensor_tensor(
                out=idx_sb[:],
                in0=len_sb[:],
                scalar=512,
                in1=idx_sb[:],
                op0=Alu.mult,
                op1=Alu.add,
            ).then_inc(idx_sem, 1)
            for r in range(ROUNDS):
                v.wait_ge(mm_sem, r + 1)
                rin = ps[:, r % NBANK, :].rearrange("p (q w m) -> p q w m", q=4, w=8, m=16)
                v.reduce_max(res[:, r * 32 : (r + 1) * 32], rin, axis=X).then_inc(
                    red_sem, 1
                )
            v.wait_ge(red_sem, ROUNDS)
            v.tensor_scalar_add(res[:], res[:], bias_sb[:, :]).then_inc(fin_sem, 1)

    return nc


def _host_inputs(char_ids, length, char_table, conv_w, conv_b):
    """Build the 8 per-core input maps (pure layout/replication on host)."""
    table_pad = np.concatenate(
        [np.asarray(char_table, np.float32), np.zeros((V, D), np.float32)], axis=0
    )
    cmat = np.zeros((128, 128), np.float32)
    cw = np.asarray(conv_w, np.float32)
    for w in range(8):
        for cc in range(16):
            for m in range(16):
                k = cc - m + 2
                if 0 <= k < KTAP:
                    cmat[w * 16 + cc, w * 16 + m] = cw[k]
    bias128 = np.full((128, 1), np.asarray(conv_b, np.float32)[0], np.float32)

    # wrapped idx layout: token i = slot*128 + (w*16 + c); wrapped[l, j] holds
    # token j*16 + l, so l = char index, j = global word index g = slot*8 + w.
    g_arr = np.arange(WORDS)
    sidx = np.tile((g_arr % S).astype(np.int16)[None, :], (128, 1))

    ids = np.asarray(char_ids, np.int16)
    lens_all = np.asarray(length, np.int16)
    in_maps = []
    for c in range(NCORES):
        idsc = ids[c * BPC : (c + 1) * BPC].reshape(WORDS, W)     # [g, char]
        idx = np.tile(np.ascontiguousarray(idsc.T), (8, 1)).astype(np.int16)
        lensc = np.tile(
            lens_all[c * BPC : (c + 1) * BPC][g_arr // S][None, :], (128, 1)
        ).astype(np.int16)
        in_maps.append(
            {
                "table": table_pad,
                "idx": idx,
                "sidx": sidx,
                "lens": lensc,
                "cmat": cmat,
                "bias": bias128,
            }
        )
    return in_maps


def _unshard(results, length):
    outs = []
    for c in range(NCORES):
        res = np.asarray(results[c]["out"])  # [128, 1024]
        data = (
            res.reshape(2, 64, ROUNDS, 4, 8)
            .transpose(2, 3, 0, 4, 1)
            .reshape(BPC, S, D)
        )
        outs.append(data)
    return np.concatenate(outs, axis=0), np.asarray(length)


def _install_ntff_hook():
    """The agent image's antenv lacks axon_hooks; recreate it so
    run_bass_kernel_spmd(trace=True) can capture NTFF profiles."""
    import types

    if "antenv.axon_hooks" in sys.modules:
        return
    import antenv

    mod = types.ModuleType("antenv.axon_hooks")
    mod._hook = None
    mod.set_axon_ntff_profile_hook = lambda h: setattr(mod, "_hook", h)
    mod.get_axon_ntff_profile_hook = lambda: mod._hook
    sys.modules["antenv.axon_hooks"] = mod
    antenv.axon_hooks = mod
    try:
        if "/root/.axon_site" not in sys.path:
            sys.path.append("/root/.axon_site")
        from trn_agent_boot.trn_boot import _ntff_profile_via_ctypes

        hook = _ntff_profile_via_ctypes("/opt/axon/libaxon_pjrt.so")
        if hook is not None:
            mod.set_axon_ntff_profile_hook(hook)
    except Exception as e:  # degrade to no tracing
        print(f"ntff hook install failed: {e}")
    # upload_artifacts needs bucket creds we don't have; skip it.
    import concourse.bass_utils as bu

    bu.upload_artifacts = lambda tmpdir: f"local://{tmpdir}"


def _run(in_maps, trace=False, tmpdir=None):
    from concourse.bass_utils import run_bass_kernel_spmd

    if "nc" not in _CACHE:
        nc = _build_nc()
        nc.compile()
        _CACHE["nc"] = nc
    nc = _CACHE["nc"]
    if trace:
        _install_ntff_hook()
    return run_bass_kernel_spmd(
        nc, in_maps, list(range(NCORES)), trace=trace, tmpdir=tmpdir
    )


def kernel(char_ids, length, char_table, conv_w, conv_b):
    in_maps = _host_inputs(char_ids, length, char_table, conv_w, conv_b)
    br = _run(in_maps, trace=False)
    data, length_out = _unshard(br.results, length)
    return data, length_out


def kernel_profiled(char_ids, length, char_table, conv_w, conv_b, tmpdir=None):
    """Like kernel() but with NTFF tracing; returns (output, exec_time_ns)."""
    in_maps = _host_inputs(char_ids, length, char_table, conv_w, conv_b)
    br = _run(in_maps, trace=True, tmpdir=tmpdir)
    data, length_out = _unshard(br.results, length)
    return (data, length_out), br.exec_time_ns


# revision 31
# speedup vs baseline: 3.1399x; 2.3825x over previous
"""Trainium2 Bass kernel for nn_CNN (char-CNN embedding + 5-tap conv + maxpool).

Computation (per reference):
  emb[b,s,w,:]  = char_table[char_ids[b,s,w], :]        (gather, 512x64 table)
  emb          *= (s < length[b])                        (word mask)
  conv[b,s,m,:] = conv_b + sum_k conv_w[k]*emb[b,s,m+k-2,:]  ('same' pad)
  out[b,s,:]    = max_m conv[b,s,m,:]
Returns (out, length).

Device strategy (pure data parallel over batch, 8 cores x 32 batches):
  - Indices of invalid words are bumped by +512 into an all-zero region of a
    [1024, 64] padded table, so masked words gather zeros -> conv = 0 ->
    max = 0 -> out = bias.  (Mask compare runs on DVE from device-side
    length/s-index tiles.)
  - Gather: per-slot indirect DMA.  Slot s_ holds 8 words x 16 chars on the
    128 partitions; one indirect_dma_start per slot pulls row idx[p, s_]
    into gbuf[p, s_, :].
  - PE computes the conv as out[d, word*16+m] = emb_slot[wc, d].T @ C[wc, .]
    with C the block-diagonal banded conv matrix (gathered tile is the
    stationary operand so output lands d-on-partitions, positions-on-free).
  - DVE reduce_max over the 16 positions (innermost free axis), then adds
    bias.  Host inverts the layout and concatenates cores.
"""

import os
import sys

os.environ.setdefault("MYCRO_LOCAL_CACHE", "1")
for _p in ("/opt/trn_rl_repo", "/root/.axon_site/_ro/trn_rl_repo"):
    if os.path.isdir(_p) and _p not in sys.path:
        sys.path.insert(0, _p)
        break

import numpy as np

B, S, W, D, V = 256, 64, 16, 64, 512
KTAP, PADW = 5, 2
NCORES = 8
BPC = B // NCORES            # 32 batches per core
WORDS = BPC * S              # 2048 words per core
TOK = WORDS * W              # 32768 gather tokens per core
SLOTS = TOK // 128           # 256 slots (8 words each)
NBANK = 8                    # psum banks in flight
ROUNDS = SLOTS // 8          # 32 psum rounds (8 slots / 64 words each)
NCHUNK = 8                   # gather chunks
SPC = SLOTS // NCHUNK        # 32 slots per gather chunk

_CACHE = {}


def _build_nc():
    from contextlib import ExitStack

    import concourse.bacc as bacc
    import concourse.bass as bass
    import concourse.mybir as mybir
    from concourse.library_config import mlp

    f32, i16 = mybir.dt.float32, mybir.dt.int16
    X = mybir.AxisListType.X
    Alu = mybir.AluOpType

    nc = bacc.Bacc("TRN2", num_swdge_queues=4)

    NW = TOK // 16  # wrapped idx columns (= words per core)
    table = nc.declare_dram_parameter("table", [2 * V, D], f32, isOutput=False)
    idx_in = nc.declare_dram_parameter("idx", [128, NW], i16, isOutput=False)
    sidx_in = nc.declare_dram_parameter("sidx", [128, NW], i16, isOutput=False)
    len_in = nc.declare_dram_parameter("lens", [128, NW], i16, isOutput=False)
    cmat_in = nc.declare_dram_parameter("cmat", [128, 128], f32, isOutput=False)
    bias_in = nc.declare_dram_parameter("bias", [128, 1], f32, isOutput=False)
    out_t = nc.declare_dram_parameter("out", [128, WORDS // 2], f32, isOutput=True)

    with (
        nc.sbuf_tensor("idx_sb", [128, NW], i16) as idx_sb,
        nc.sbuf_tensor("sidx_sb", [128, NW], i16) as sidx_sb,
        nc.sbuf_tensor("len_sb", [128, NW], i16) as len_sb,
        nc.sbuf_tensor("cmat_sb", [128, 128], f32) as cmat_sb,
        nc.sbuf_tensor("bias_sb", [128, 1], f32) as bias_sb,
        nc.sbuf_tensor("gbuf", [128, SLOTS, D], f32) as gbuf,
        nc.sbuf_tensor("res", [128, WORDS // 2], f32) as res,
        nc.psum_tensor("ps", [128, NBANK, 512], f32) as ps,
        nc.semaphore("in_sem") as in_sem,
        nc.semaphore("msk_sem") as msk_sem,
        nc.semaphore("idx_sem") as idx_sem,
        nc.semaphore("mm_sem") as mm_sem,
        nc.semaphore("red_sem") as red_sem,
        nc.semaphore("fin_sem") as fin_sem,
        ExitStack() as stack,
        nc.Block() as block,
    ):
        gat_sems = [
            stack.enter_context(nc.semaphore(f"gat{c}")) for c in range(NCHUNK + 1)
        ]

        @block.sync
        def _(sp):
            sp.dma_start(out=idx_sb[:], in_=idx_in[:]).then_inc(in_sem, 16)
            sp.dma_start(out=sidx_sb[:], in_=sidx_in[:]).then_inc(in_sem, 16)
            sp.dma_start(out=len_sb[:], in_=len_in[:]).then_inc(in_sem, 16)
            sp.dma_start(out=cmat_sb[:], in_=cmat_in[:]).then_inc(in_sem, 16)
            sp.dma_start(out=bias_sb[:], in_=bias_in[:]).then_inc(in_sem, 16)
            sp.wait_ge(fin_sem, 1)
            sp.dma_start(out=out_t[:], in_=res[:]).then_inc(in_sem, 16)
            sp.wait_ge(in_sem, 96)

        IDX_PC = SPC * 128  # indices per gather chunk

        @block.gpsimd
        def _(g):
            g.load_library(mlp)
            g.wait_ge(idx_sem, 1)
            for c in range(NCHUNK):
                g.dma_gather(
                    gbuf[:, c * SPC : (c + 1) * SPC, :],
                    table[:, :],
                    idx_sb[:, c * (IDX_PC // 16) : (c + 1) * (IDX_PC // 16)],
                    IDX_PC,
                    IDX_PC,
                    D,
                    single_packet=False,
                    queue_num=c % 4,
                ).then_inc(gat_sems[c], 16)

        @block.tensor
        def _(t):
            t.wait_ge(in_sem, 80)
            for r in range(ROUNDS):
                c = r // (ROUNDS // NCHUNK)
                t.wait_ge(gat_sems[c], 16)
                if r >= NBANK:
                    t.wait_ge(red_sem, r - NBANK + 1)
                bank = ps[:, r % NBANK, :]
                mm = None
                for q in range(4):
                    for par in range(2):
                        s_ = r * 8 + q * 2 + par
                        mm = nc.tensor.matmul(
                            bank[par * 64 : (par + 1) * 64, q * 128 : (q + 1) * 128],
                            gbuf[:, s_, :],
                            cmat_sb[:, :],
                            start=True,
                            stop=True,
                        )
                mm.then_inc(mm_sem, 1)

        @block.vector
        def _(v):
            v.wait_ge(in_sem, 80)
            # ge = (s_of_word >= length_of_word)  (1/0), then idx += 512*ge
            # -> masked words gather the all-zero rows [512, 1024).
            v.tensor_tensor(
                out=len_sb[:], in0=sidx_sb[:], in1=len_sb[:], op=Alu.is_ge
            ).then_inc(msk_sem, 1)
            v.wait_ge(msk_sem, 1)
            v.scalar_tensor_tensor(
                out=idx_sb[:],
                in0=len_sb[:],
                scalar=512,
                in1=idx_sb[:],
                op0=Alu.mult,
                op1=Alu.add,
            ).then_inc(idx_sem, 1)
            for r in range(ROUNDS):
                v.wait_ge(mm_sem, r + 1)
                rin = ps[:, r % NBANK, :].rearrange("p (q w m) -> p q w m", q=4, w=8, m=16)
                v.reduce_max(res[:, r * 32 : (r + 1) * 32], rin, axis=X).then_inc(
                    red_sem, 1
                )
            v.wait_ge(red_sem, ROUNDS)
            v.tensor_scalar_add(res[:], res[:], bias_sb[:, :]).then_inc(fin_sem, 1)

    return nc


def _host_inputs(char_ids, length, char_table, conv_w, conv_b):
    """Build the 8 per-core input maps (pure layout/replication on host)."""
    table_pad = np.concatenate(
        [np.asarray(char_table, np.float32), np.zeros((V, D), np.float32)], axis=0
    )
    cmat = np.zeros((128, 128), np.float32)
    cw = np.asarray(conv_w, np.float32)
    for w in range(8):
        for cc in range(16):
            for m in range(16):
                k = cc - m + 2
                if 0 <= k < KTAP:
                    cmat[w * 16 + cc, w * 16 + m] = cw[k]
    bias128 = np.full((128, 1), np.asarray(conv_b, np.float32)[0], np.float32)

    # wrapped idx layout: token i = slot*128 + (w*16 + c); wrapped[l, j] holds
    # token j*16 + l, so l = char index, j = global word index g = slot*8 + w.
    g_arr = np.arange(WORDS)
    sidx = np.tile((g_arr % S).astype(np.int16)[None, :], (128, 1))

    ids = np.asarray(char_ids, np.int16)
    lens_all = np.asarray(length, np.int16)
    in_maps = []
    for c in range(NCORES):
        idsc = ids[c * BPC : (c + 1) * BPC].reshape(WORDS, W)     # [g, char]
        idx = np.tile(np.ascontiguousarray(idsc.T), (8, 1)).astype(np.int16)
        lensc = np.tile(
            lens_all[c * BPC : (c + 1) * BPC][g_arr // S][None, :], (128, 1)
        ).astype(np.int16)
        in_maps.append(
            {
                "table": table_pad,
                "idx": idx,
                "sidx": sidx,
                "lens": lensc,
                "cmat": cmat,
                "bias": bias128,
            }
        )
    return in_maps


def _unshard(results, length):
    outs = []
    for c in range(NCORES):
        res = np.asarray(results[c]["out"])  # [128, 1024]
        data = (
            res.reshape(2, 64, ROUNDS, 4, 8)
            .transpose(2, 3, 0, 4, 1)
            .reshape(BPC, S, D)
        )
        outs.append(data)
    return np.concatenate(outs, axis=0), np.asarray(length)


def _install_ntff_hook():
    """The agent image's antenv lacks axon_hooks; recreate it so
    run_bass_kernel_spmd(trace=True) can capture NTFF profiles."""
    import types

    if "antenv.axon_hooks" in sys.modules:
        return
    import antenv

    mod = types.ModuleType("antenv.axon_hooks")
    mod._hook = None
    mod.set_axon_ntff_profile_hook = lambda h: setattr(mod, "_hook", h)
    mod.get_axon_ntff_profile_hook = lambda: mod._hook
    sys.modules["antenv.axon_hooks"] = mod
    antenv.axon_hooks = mod
    try:
        if "/root/.axon_site" not in sys.path:
            sys.path.append("/root/.axon_site")
        from trn_agent_boot.trn_boot import _ntff_profile_via_ctypes

        hook = _ntff_profile_via_ctypes("/opt/axon/libaxon_pjrt.so")
        if hook is not None:
            mod.set_axon_ntff_profile_hook(hook)
    except Exception as e:  # degrade to no tracing
        print(f"ntff hook install failed: {e}")
    # upload_artifacts needs bucket creds we don't have; skip it.
    import concourse.bass_utils as bu

    bu.upload_artifacts = lambda tmpdir: f"local://{tmpdir}"


def _run(in_maps, trace=False, tmpdir=None):
    from concourse.bass_utils import run_bass_kernel_spmd

    if "nc" not in _CACHE:
        nc = _build_nc()
        nc.compile()
        _CACHE["nc"] = nc
    nc = _CACHE["nc"]
    if trace:
        _install_ntff_hook()
    return run_bass_kernel_spmd(
        nc, in_maps, list(range(NCORES)), trace=trace, tmpdir=tmpdir
    )


def kernel(char_ids, length, char_table, conv_w, conv_b):
    in_maps = _host_inputs(char_ids, length, char_table, conv_w, conv_b)
    br = _run(in_maps, trace=False)
    data, length_out = _unshard(br.results, length)
    return data, length_out


def kernel_profiled(char_ids, length, char_table, conv_w, conv_b, tmpdir=None):
    """Like kernel() but with NTFF tracing; returns (output, exec_time_ns)."""
    in_maps = _host_inputs(char_ids, length, char_table, conv_w, conv_b)
    br = _run(in_maps, trace=True, tmpdir=tmpdir)
    data, length_out = _unshard(br.results, length)
    return (data, length_out), br.exec_time_ns
